# revision 1
# baseline (speedup 1.0000x reference)
"""Multi-head attention (B=2, S=2048, D=1024, H=16) on 8 Trainium2 NeuronCores.

Sharding: tensor-parallel over heads x data-parallel over batch.
  core c -> batch g = c // 4, head group r = c % 4 (global heads 4r..4r+3).
Each core computes qkv projections for its 4 heads (w_qkv column slices),
attention for those heads over the full sequence of its batch, then an
AllToAll inside each 4-core group converts the head-sharded attention
output into a sequence-sharded one, so every core runs the full output
projection for its own 512 sequence rows. Host-side work is only
slicing/transposition of inputs and concatenation of outputs.

Device algorithm (per core):
  qT, kT   [256, 2048]  (partition = head*64+hd, free = seq)
  V        [2048, 256]  (partition = seq, free = head*64+hd)
  per (sq-chunk 1024, head pair):
    per sk-tile (128 keys):
      scoresT[sk, sq] = kT.T @ qT   (two K=64 matmuls row-packed in PE)
      pT = exp(0.125 * scoresT)     (ACT, PSUM -> SBUF)
      rowsum_acc += pT              (DVE)
      outT += V.T-slice @ pT        (PSUM accumulate, col-packed pairs)
    rowsum broadcast = ones[128,128].T @ rowsum_acc  (reduce over sk partitions)
    outT *= 1/rowsum  (DVE reciprocal + mul) -> attn_outT [256, 2048]
  Per (pair, sq-chunk): 4-rank AllGather of the finished attn_outT slice
  (first three overlap with remaining attention compute), then each core
  projects its own 512 sequence rows: out = attn_outT.T @ w_proj + b_proj.
"""

import os
import sys

import numpy as np

try:
    import ml_dtypes
    BF16_NP = ml_dtypes.bfloat16
except ImportError:  # pragma: no cover
    BF16_NP = None

for _p in ("/opt/trn_rl_repo",):
    if os.path.isdir(_p) and _p not in sys.path:
        sys.path.append(_p)

import concourse.bass as bass  # noqa: E402
import concourse.mybir as mybir  # noqa: E402
import concourse.tile as tile  # noqa: E402
from concourse import bacc  # noqa: E402
from concourse.bass_utils import run_bass_kernel_spmd  # noqa: E402

B, S, D = 2, 2048, 1024
H, HD = 16, 64
N_CORES = 8
GROUP = 4  # cores per batch group
LH = H // GROUP  # local heads per core = 4
LHD = LH * HD  # 256 local head dims
S_OWN = S // GROUP  # 512 sequence rows owned for the projection
FP32 = mybir.dt.float32
FP32R = mybir.dt.float32r
BF16 = mybir.dt.bfloat16

SQ_CHUNK = 1024  # query-range processed per inner attention pass
N_SQ = S // SQ_CHUNK  # 2
N_SK = S // 128  # 16 key tiles
N_KT = D // 128  # 8 contraction tiles for the projections

_compiled = None
_ONES = np.ones((128, 128), dtype=np.float32)
_ONES16 = None  # set lazily (needs ml_dtypes)


def _build():
    nc = bacc.Bacc(
        "TRN2", target_bir_lowering=False, debug=False, num_devices=N_CORES
    )

    xT_d = nc.dram_tensor("xT", [D, S], BF16, kind="ExternalInput")
    wq_d = nc.dram_tensor("wq", [D, LHD], BF16, kind="ExternalInput")
    wk_d = nc.dram_tensor("wk", [D, LHD], BF16, kind="ExternalInput")
    wv_d = nc.dram_tensor("wv", [D, LHD], BF16, kind="ExternalInput")
    wp_d = nc.dram_tensor("wp", [D, D], BF16, kind="ExternalInput")
    bq_d = nc.dram_tensor("bq", [LHD, 1], FP32, kind="ExternalInput")
    bk_d = nc.dram_tensor("bk", [LHD, 1], FP32, kind="ExternalInput")
    bv_d = nc.dram_tensor("bv", [128, LHD], FP32, kind="ExternalInput")
    bp_d = nc.dram_tensor("bp", [128, D], FP32, kind="ExternalInput")
    ones_d = nc.dram_tensor("ones", [128, 128], FP32R, kind="ExternalInput")
    ones16_d = nc.dram_tensor("ones16", [128, 8], BF16, kind="ExternalInput")
    out_d = nc.dram_tensor("out", [S_OWN, D], FP32, kind="ExternalOutput")

    # Two AllGathers (one per head pair) inside each 4-core batch group;
    # the first runs while the second pair's attention still computes.
    ag_in = [[nc.dram_tensor(f"ag_in{p}_{c}", [128, SQ_CHUNK], BF16)
              for c in range(N_SQ)] for p in range(2)]
    ag_out = [nc.dram_tensor(f"ag_out{p}", [N_SQ * GROUP * 128, SQ_CHUNK], BF16)
              for p in range(2)]
    groups = [[0, 1, 2, 3], [4, 5, 6, 7]]

    with tile.TileContext(nc) as tc:
        import contextlib

        with contextlib.ExitStack() as stk:
            # ---- long-lived pools -------------------------------------
            qk_pool = stk.enter_context(tc.tile_pool(name="qk", bufs=1))
            v_pool = stk.enter_context(tc.tile_pool(name="v", bufs=1))
            ao_pool = stk.enter_context(tc.tile_pool(name="ao", bufs=1))
            const_pool = stk.enter_context(tc.tile_pool(name="const", bufs=1))
            wp_pool = stk.enter_context(tc.tile_pool(name="wp", bufs=1))

            qT = [qk_pool.tile([128, S], BF16, name=f"qT{j}", tag=f"qT{j}") for j in range(2)]
            kT = [qk_pool.tile([128, S], BF16, name=f"kT{j}", tag=f"kT{j}") for j in range(2)]
            vp = [v_pool.tile([128, LH * 65], BF16, name=f"v{m}", tag=f"v{m}") for m in range(16)]
            aoT = [ao_pool.tile([128, S], BF16, name=f"ao{p}", tag=f"ao{p}") for p in range(2)]

            ones_t = const_pool.tile([128, 128], FP32R, tag="ones")
            nc.sync.dma_start(ones_t[:], ones_d.ap())
            bq_t = [const_pool.tile([128, 1], FP32, name=f"bq{j}", tag=f"bq{j}")
                    for j in range(2)]
            bk_t = [const_pool.tile([128, 1], FP32, name=f"bk{j}", tag=f"bk{j}")
                    for j in range(2)]
            bv_t = const_pool.tile([128, LHD], FP32, tag="bv")
            bp_t = const_pool.tile([128, D], FP32, tag="bp")
            for j in range(2):
                jsl = slice(j * 128, (j + 1) * 128)
                nc.sync.dma_start(bq_t[j][:], bq_d.ap()[jsl, :])
                nc.sync.dma_start(bk_t[j][:], bk_d.ap()[jsl, :])
            nc.sync.dma_start(bv_t[:], bv_d.ap())
            nc.sync.dma_start(bp_t[:], bp_d.ap())

            wp_t = [wp_pool.tile([128, D], BF16, name=f"wp{k}", tag=f"wp{k}") for k in range(N_KT)]

            # ---- PE warm-up: keep the HAM clock gate open during the
            # input DMA ramp (dummy matmuls into a scratch psum bank)
            with tc.tile_pool(name="warm", bufs=1, space="PSUM") as warm_pool:
                wps = warm_pool.tile([128, 128], FP32, tag="warm")
                for w in range(32):
                    nc.tensor.matmul(
                        wps[:],
                        ones_t[:],
                        ones_t[:],
                        start=True, stop=True,
                        skip_group_check=True,
                    )

            # ---- phase A: qkv projections -----------------------------
            with (
                tc.tile_pool(name="x", bufs=1) as x_pool,
                tc.tile_pool(name="w", bufs=1) as w_pool,
                tc.tile_pool(name="psA", bufs=1, space="PSUM") as psA,
            ):
                x_t = [x_pool.tile([128, S], BF16, name=f"x{k}", tag=f"x{k}") for k in range(N_KT)]
                wq_t = [w_pool.tile([128, LHD], BF16, name=f"wq{k}", tag=f"wq{k}") for k in range(N_KT)]
                wk_t = [w_pool.tile([128, LHD], BF16, name=f"wk{k}", tag=f"wk{k}") for k in range(N_KT)]
                wv_t = [w_pool.tile([128, LHD], BF16, name=f"wv{k}", tag=f"wv{k}") for k in range(N_KT)]
                # interleave x/w loads k-major and fan out across four DMA
                # queues so the first contraction tiles land quickly
                dma_engs = [nc.sync, nc.scalar]
                for k in range(N_KT):
                    sl = slice(k * 128, (k + 1) * 128)
                    eng = dma_engs[k % 2]
                    eng.dma_start(x_t[k][:], xT_d.ap()[sl, :])
                    eng2 = dma_engs[(k + 1) % 2]
                    eng2.dma_start(wq_t[k][:], wq_d.ap()[sl, :])
                    eng2.dma_start(wk_t[k][:], wk_d.ap()[sl, :])
                    eng2.dma_start(wv_t[k][:], wv_d.ap()[sl, :])

                # qT / kT / V in PSUM waves, contraction k outermost so the
                # PE follows the xT tiles as they stream in from HBM
                for j in range(2):
                    ps_q = [psA.tile([128, 512], FP32, name=f"psq{j}{sc}", tag=f"psA{sc}") for sc in range(4)]
                    ps_k = [psA.tile([128, 512], FP32, name=f"psk{j}{sc}", tag=f"psA{sc+4}") for sc in range(4)]
                    for k in range(N_KT):
                        for sc in range(4):
                            ssl = slice(sc * 512, (sc + 1) * 512)
                            nc.tensor.matmul(
                                ps_q[sc][:],
                                wq_t[k][:, j * 128 : (j + 1) * 128],
                                x_t[k][:, ssl],
                                start=(k == 0), stop=(k == N_KT - 1),
                            )
                            nc.tensor.matmul(
                                ps_k[sc][:],
                                wk_t[k][:, j * 128 : (j + 1) * 128],
                                x_t[k][:, ssl],
                                start=(k == 0), stop=(k == N_KT - 1),
                            )
                    for sc in range(4):
                        ssl = slice(sc * 512, (sc + 1) * 512)
                        nc.vector.tensor_scalar(
                            qT[j][:, ssl], ps_q[sc][:], bq_t[j][:], None,
                            mybir.AluOpType.add,
                        )
                        nc.vector.tensor_scalar(
                            kT[j][:, ssl], ps_k[sc][:], bk_t[j][:], None,
                            mybir.AluOpType.add,
                        )
                # V: [s-tile 128, 256] = x.T @ wv, two waves of 8 m-tiles
                for wave in range(2):
                    ps_v = [psA.tile([128, LHD], FP32, name=f"psv{wave}{i}", tag=f"psA{i}") for i in range(8)]
                    for k in range(N_KT):
                        for i in range(8):
                            m = wave * 8 + i
                            nc.tensor.matmul(
                                ps_v[i][:],
                                x_t[k][:, m * 128 : (m + 1) * 128],
                                wv_t[k][:],
                                start=(k == 0), stop=(k == N_KT - 1),
                            )
                    for i in range(8):
                        m = wave * 8 + i
                        for h in range(LH):
                            nc.vector.tensor_tensor(
                                vp[m][:, 65 * h : 65 * h + 64],
                                ps_v[i][:, 64 * h : 64 * h + 64],
                                bv_t[:, 64 * h : 64 * h + 64],
                                mybir.AluOpType.add,
                            )
                        nc.sync.dma_start(vp[m][:, 64::65], ones16_d.ap()[:, 0:LH])

            # weight prefetch for phase D (scheduler fills DMA gaps)
            for k in range(N_KT):
                nc.sync.dma_start(wp_t[k][:], wp_d.ap()[k * 128 : (k + 1) * 128, :])

            at_pool = stk.enter_context(tc.tile_pool(name="at", bufs=1))
            at_t = [at_pool.tile([128, S_OWN], BF16, name=f"at{k}", tag=f"at{k}")
                    for k in range(N_KT)]
            pid = nc.gpsimd.partition_id()
            rank = pid % GROUP
            col0 = (rank % 2) * S_OWN

            def load_at(k):
                row0 = (rank // 2) * 512 + 128 * (k // 2)
                nc.gpsimd.dma_start(
                    at_t[k][:],
                    ag_out[k % 2].ap()[bass.ds(row0, 128), bass.ds(col0, S_OWN)],
                )

            # ---- phase B: attention -----------------------------------
            with (
                tc.tile_pool(name="p", bufs=4) as p_pool,
                tc.tile_pool(name="rr", bufs=4) as rr_pool,
                tc.tile_pool(name="rcp", bufs=2) as rcp_pool,
                tc.tile_pool(name="psc", bufs=2, space="PSUM") as ps_sc,
                tc.tile_pool(name="pacc", bufs=1, space="PSUM") as ps_acc,
            ):
                for p in range(2):  # head pair: local heads 2p, 2p+1
                    for cq in range(N_SQ):
                        qsl = slice(cq * SQ_CHUNK, (cq + 1) * SQ_CHUNK)
                        # row 64 of each acc collects the softmax denominator
                        # via the ones column appended to V
                        acc_a = ps_acc.tile([65, SQ_CHUNK], FP32, tag="acca")
                        acc_b = ps_acc.tile([65, SQ_CHUNK], FP32, tag="accb")
                        for t in range(N_SK):
                            tsl = slice(t * 128, (t + 1) * 128)
                            sca = ps_sc.tile([128, SQ_CHUNK], FP32, tag="sc")
                            scb = ps_sc.tile([128, SQ_CHUNK], FP32, tag="sc")
                            for u in range(SQ_CHUNK // 512):
                                usl = slice(u * 512, (u + 1) * 512)
                                gsl = slice(cq * SQ_CHUNK + u * 512,
                                            cq * SQ_CHUNK + (u + 1) * 512)
                                nc.tensor.matmul(
                                    sca[:, usl],
                                    kT[p][0:64, tsl],
                                    qT[p][0:64, gsl],
                                    start=True, stop=True,
                                    tile_position=(0, 0),
                                )
                                nc.tensor.matmul(
                                    scb[:, usl],
                                    kT[p][64:128, tsl],
                                    qT[p][64:128, gsl],
                                    start=True, stop=True,
                                    tile_position=(64, 0),
                                )
                            pa = p_pool.tile([128, SQ_CHUNK], BF16, tag="pt")
                            pb = p_pool.tile([128, SQ_CHUNK], BF16, tag="pt")
                            nc.scalar.activation(
                                pa[:], sca[:],
                                mybir.ActivationFunctionType.Exp, scale=0.125,
                            )
                            nc.scalar.activation(
                                pb[:], scb[:],
                                mybir.ActivationFunctionType.Exp, scale=0.125,
                            )
                            for u in range(SQ_CHUNK // 512):
                                usl = slice(u * 512, (u + 1) * 512)
                                nc.tensor.matmul(
                                    acc_a[:, usl],
                                    vp[t][:, 65 * (2 * p) : 65 * (2 * p) + 65],
                                    pa[:, usl],
                                    start=(t == 0), stop=(t == N_SK - 1),
                                )
                                nc.tensor.matmul(
                                    acc_b[:, usl],
                                    vp[t][:, 65 * (2 * p + 1) : 65 * (2 * p + 1) + 65],
                                    pb[:, usl],
                                    start=(t == 0), stop=(t == N_SK - 1),
                                )
                        # normalize: 1/rowsum broadcast across the 64 head dims
                        for acc, half in ((acc_a, 0), (acc_b, 1)):
                            rrow = rr_pool.tile([1, SQ_CHUNK], FP32R, tag="rrow")
                            nc.vector.tensor_copy(rrow[:], acc[64:65, :])
                            rb = ps_sc.tile([64, SQ_CHUNK], FP32, tag="sc")
                            for u in range(SQ_CHUNK // 512):
                                usl = slice(u * 512, (u + 1) * 512)
                                nc.tensor.matmul(
                                    rb[:, usl], ones_t[0:1, 0:64], rrow[:, usl],
                                    start=True, stop=True,
                                )
                            rc = rcp_pool.tile([64, SQ_CHUNK], FP32, tag="rc")
                            nc.vector.reciprocal_approx_fast(rc[:], rb[:])
                            nc.vector.tensor_tensor(
                                aoT[p][64 * half : 64 * half + 64, qsl],
                                acc[0:64, :], rc[:],
                                mybir.AluOpType.mult,
                            )
                        # gather this (pair, sq-chunk) while compute continues
                        nc.sync.dma_start(ag_in[p][cq].ap(), aoT[p][:, qsl])
                        nc.gpsimd.collective_compute(
                            "AllGather",
                            mybir.AluOpType.bypass,
                            replica_groups=groups,
                            ins=[ag_in[p][cq].ap()],
                            outs=[ag_out[p].ap()[cq * 512 : (cq + 1) * 512, :]],
                        )
                        if p == 1:
                            # pair-0 loads go after pair-1's first collective
                            # trigger (their wait is then already satisfied and
                            # cannot stall the queue); pair-1 loads at the end
                            for k in range(cq, N_KT, 2):
                                load_at(k)


            # ---- phase D: output projection on own 512 rows -----------
            with (
                tc.tile_pool(name="outp", bufs=4) as out_pool,
                tc.tile_pool(name="psD", bufs=1, space="PSUM") as psD,
            ):
                # logical head-row block k lives in ag_out[k%2]; the
                # gathered rows are stacked [sq-chunk][group-rank][128],
                # and this core's sequence window picks chunk (rank//2)
                # at column offset (rank%2)*512
                # two-pass projection: every tile's even-k contributions
                # (available after the early pair-0 gathers) run first and can
                # overlap the final AllGather; the odd-k half follows
                tiles = [(m, nb) for m in range(S_OWN // 128) for nb in range(2)]
                ps_all = {}
                for m, nb in tiles:
                    ps_all[(m, nb)] = psD.tile(
                        [128, 512], FP32, name=f"psD{m}{nb}", tag=f"psD{m}{nb}"
                    )
                for ks, first, last in (((0, 2, 4, 6), True, False),
                                        ((1, 3, 5, 7), False, True)):
                    for m, nb in tiles:
                        msl = slice(m * 128, (m + 1) * 128)
                        nsl = slice(nb * 512, (nb + 1) * 512)
                        for ki, k in enumerate(ks):
                            nc.tensor.matmul(
                                ps_all[(m, nb)][:],
                                at_t[k][:, msl],
                                wp_t[k][:, nsl],
                                start=(first and ki == 0),
                                stop=(last and ki == 3),
                            )
                for m, nb in tiles:
                    msl = slice(m * 128, (m + 1) * 128)
                    nsl = slice(nb * 512, (nb + 1) * 512)
                    ot = out_pool.tile([128, 512], FP32, tag="ot")
                    nc.vector.tensor_tensor(
                        ot[:], ps_all[(m, nb)][:], bp_t[:, nsl], mybir.AluOpType.add
                    )
                    nc.sync.dma_start(out_d.ap()[msl, nsl], ot[:])

    nc.compile()
    return nc


def _get_program():
    global _compiled
    if _compiled is None:
        _compiled = _build()
    return _compiled


def _make_in_maps(x, w_qkv, b_qkv, w_proj, b_proj):
    x = np.asarray(x, dtype=np.float32)
    w_qkv = np.asarray(w_qkv, dtype=np.float32)
    b_qkv = np.asarray(b_qkv, dtype=np.float32)
    w_proj = np.asarray(w_proj, dtype=np.float32)
    b_proj = np.asarray(b_proj, dtype=np.float32)

    global _ONES16
    if _ONES16 is None:
        _ONES16 = np.ones((128, 8), dtype=BF16_NP)
    wp16 = w_proj.astype(BF16_NP)
    bp_b = np.ascontiguousarray(np.broadcast_to(b_proj.reshape(1, D), (128, D)))
    in_maps = []
    for c in range(N_CORES):
        g, r = c // GROUP, c % GROUP
        xT = np.ascontiguousarray(x[g].T)
        in_maps.append(
            {
                "xT": xT.astype(BF16_NP),
                "wq": w_qkv[:, 0 * D + r * LHD : 0 * D + (r + 1) * LHD].astype(BF16_NP),
                "wk": w_qkv[:, 1 * D + r * LHD : 1 * D + (r + 1) * LHD].astype(BF16_NP),
                "wv": w_qkv[:, 2 * D + r * LHD : 2 * D + (r + 1) * LHD].astype(BF16_NP),
                "wp": wp16,
                "bq": np.ascontiguousarray(b_qkv[0 * D + r * LHD : 0 * D + (r + 1) * LHD].reshape(LHD, 1)),
                "bk": np.ascontiguousarray(b_qkv[1 * D + r * LHD : 1 * D + (r + 1) * LHD].reshape(LHD, 1)),
                "bv": np.ascontiguousarray(
                    np.broadcast_to(
                        b_qkv[2 * D + r * LHD : 2 * D + (r + 1) * LHD].reshape(1, LHD),
                        (128, LHD),
                    )
                ),
                "bp": bp_b,
                "ones": _ONES,
                "ones16": _ONES16,
            }
        )
    return in_maps


def _assemble(results):
    out = np.empty((B, S, D), dtype=np.float32)
    for c in range(N_CORES):
        g, r = c // GROUP, c % GROUP
        out[g, r * S_OWN : (r + 1) * S_OWN, :] = results[c]["out"]
    return out


def kernel(x, w_qkv, b_qkv, w_proj, b_proj):
    nc = _get_program()
    in_maps = _make_in_maps(x, w_qkv, b_qkv, w_proj, b_proj)
    res = run_bass_kernel_spmd(nc, in_maps, list(range(N_CORES)))
    return _assemble(res.results)



# revision 3
# speedup vs baseline: 1.0404x; 1.0404x over previous
"""Multi-head attention (B=2, S=2048, D=1024, H=16) on 8 Trainium2 NeuronCores.

Sharding: data-parallel over batch (groups of 4 cores) x sequence-parallel
attention inside each group.
  core c -> batch g = c // 4, sequence block r = c % 4 (rows r*512..r*512+512).

Per core:
  phase A (projections, all N=512 K=128 matmuls):
    kT_own [256, 2048]  = wk_own.T @ x.T   (K for this core's 4 heads, all 2048 keys)
    V_own  [512, 1024]  = x_own @ wv       (V for this core's 512 rows, all heads)
    qT_own [1024, 512]  = wq.T @ x_own.T   (Q for this core's 512 queries, all heads)
  Two early AllGathers inside each 4-core group (overlap the q projection and
  the first attention pairs): kT_own -> kT_full [1024, 2048] (partition-concat
  = head-major dims) and V_own -> V_full [2048, 1024] (row concat).
  phase B (attention, all 16 heads x own 512 queries x 2048 keys):
    per head pair p, per key-tile pair (t, t+1):
      scoresT [128 keys, 1024] = kT tile.T @ q      (q zero-padded per head so
        K=128 and consecutive matmuls share the stationary operand)
      p = exp(0.125 * scoresT)                       (one N=1024 ACT op per head)
      accT [65, 512] += V-slice(+ones col).T @ p     (PSUM accumulate over tiles;
        row 64 collects the softmax denominator)
    normalize via ones-matmul broadcast + reciprocal -> aoT [1024, 512]
  phase C (local output projection, no collective on the tail):
    out_own [512, 1024] = aoT.T @ w_proj + b_proj
Host-side work is only slicing/transposition of inputs and concatenation of
outputs.
"""

import os
import sys

import numpy as np

try:
    import ml_dtypes

    BF16_NP = ml_dtypes.bfloat16
except ImportError:  # pragma: no cover
    BF16_NP = None

for _p in ("/opt/trn_rl_repo",):
    if os.path.isdir(_p) and _p not in sys.path:
        sys.path.append(_p)

import concourse.bass as bass  # noqa: E402
import concourse.mybir as mybir  # noqa: E402
import concourse.tile as tile  # noqa: E402
from concourse import bacc  # noqa: E402
from concourse.bass_utils import run_bass_kernel_spmd  # noqa: E402

B, S, D = 2, 2048, 1024
H, HD = 16, 64
N_CORES = 8
GROUP = 4
LH = H // GROUP  # heads whose K this core computes = 4
LHD = LH * HD  # 256
S_OWN = S // GROUP  # 512 own sequence rows / queries
N_KT = D // 128  # 8 contraction tiles
N_SK = S // 128  # 16 key tiles
PAIRS = H // 2  # 8 head pairs

FP32 = mybir.dt.float32
FP32R = mybir.dt.float32r
BF16 = mybir.dt.bfloat16

_compiled = None
_ONES = np.ones((128, 128), dtype=np.float32)
_ONES16 = None


def _build():
    nc = bacc.Bacc(
        "TRN2", target_bir_lowering=False, debug=False, num_devices=N_CORES
    )

    xT_d = nc.dram_tensor("xT", [D, S], BF16, kind="ExternalInput")
    xq_d = nc.dram_tensor("xq", [D, S_OWN], BF16, kind="ExternalInput")
    wq_d = nc.dram_tensor("wq", [D, D], BF16, kind="ExternalInput")
    wk_d = nc.dram_tensor("wk", [D, LHD], BF16, kind="ExternalInput")
    wv_d = nc.dram_tensor("wv", [D, D], BF16, kind="ExternalInput")
    wp_d = nc.dram_tensor("wp", [D, D], BF16, kind="ExternalInput")
    bqa_d = nc.dram_tensor("bqa", [128, PAIRS], FP32, kind="ExternalInput")
    bqb_d = nc.dram_tensor("bqb", [128, PAIRS], FP32, kind="ExternalInput")
    maska_d = nc.dram_tensor("maska", [128, 1], FP32, kind="ExternalInput")
    maskb_d = nc.dram_tensor("maskb", [128, 1], FP32, kind="ExternalInput")
    bk_d = nc.dram_tensor("bk", [LHD, 1], FP32, kind="ExternalInput")
    bv_d = nc.dram_tensor("bv", [128, D], FP32, kind="ExternalInput")
    bp_d = nc.dram_tensor("bp", [128, D], FP32, kind="ExternalInput")
    ones_d = nc.dram_tensor("ones", [128, 128], FP32R, kind="ExternalInput")
    ones16_d = nc.dram_tensor("ones16", [128, H], BF16, kind="ExternalInput")
    out_d = nc.dram_tensor("out", [S_OWN, D], FP32, kind="ExternalOutput")

    agk_in = nc.dram_tensor("agk_in", [LHD, S], BF16)
    agk_out = nc.dram_tensor("agk_out", [D, S], BF16)
    agv_in = nc.dram_tensor("agv_in", [S_OWN, D], BF16)
    agv_out = nc.dram_tensor("agv_out", [S, D], BF16)
    groups = [[0, 1, 2, 3], [4, 5, 6, 7]]

    with tile.TileContext(nc) as tc:
        import contextlib

        with contextlib.ExitStack() as stk:
            # ---- persistent pools --------------------------------------
            w_pool = stk.enter_context(tc.tile_pool(name="w", bufs=1))
            kt_pool = stk.enter_context(tc.tile_pool(name="kt", bufs=1))
            vp_pool = stk.enter_context(tc.tile_pool(name="vp", bufs=1))
            q_pool = stk.enter_context(tc.tile_pool(name="q", bufs=1))
            ao_pool = stk.enter_context(tc.tile_pool(name="ao", bufs=1))
            const_pool = stk.enter_context(tc.tile_pool(name="const", bufs=1))

            # big 1024-wide weight tiles: wq(8) + wv(8) live together in
            # phase A; wp(8) rotates into wq's slots afterwards
            wq_t = [w_pool.tile([128, D], BF16, name=f"wq{k}", tag="w1024", bufs=16)
                    for k in range(N_KT)]
            wv_t = [w_pool.tile([128, D], BF16, name=f"wv{k}", tag="w1024", bufs=16)
                    for k in range(N_KT)]
            wk_t = [w_pool.tile([128, LHD], BF16, name=f"wk{k}", tag=f"wk{k}")
                    for k in range(N_KT)]

            kT = [kt_pool.tile([128, S], BF16, name=f"kT{p}", tag=f"kT{p}")
                  for p in range(PAIRS)]
            vp = [vp_pool.tile([128, H, HD + 1], BF16, name=f"vp{m}", tag=f"vp{m}")
                  for m in range(N_SK)]
            qA = [q_pool.tile([128, S_OWN], BF16, name=f"qA{p}", tag=f"qA{p}")
                  for p in range(PAIRS)]
            qB = [q_pool.tile([128, S_OWN], BF16, name=f"qB{p}", tag=f"qB{p}")
                  for p in range(PAIRS)]
            aoT = [ao_pool.tile([128, S_OWN], BF16, name=f"ao{p}", tag=f"ao{p}")
                   for p in range(PAIRS)]

            ones_t = const_pool.tile([128, 128], FP32R, tag="ones")
            ones16_t = const_pool.tile([128, H], BF16, tag="ones16")
            bqa_t = const_pool.tile([128, PAIRS], FP32, tag="bqa")
            bqb_t = const_pool.tile([128, PAIRS], FP32, tag="bqb")
            maska_t = const_pool.tile([128, 1], FP32, tag="maska")
            maskb_t = const_pool.tile([128, 1], FP32, tag="maskb")
            bk_t = [const_pool.tile([128, 1], FP32, name=f"bk{j}", tag=f"bk{j}")
                    for j in range(2)]
            bv_t = const_pool.tile([128, D], FP32, tag="bv")
            bp_t = const_pool.tile([128, D], FP32, tag="bp")
            actw_t = const_pool.tile([128, 1], FP32, tag="actw")

            # const DMAs on the gpsimd queue (idle early)
            nc.gpsimd.dma_start(ones_t[:], ones_d.ap())
            nc.gpsimd.dma_start(ones16_t[:], ones16_d.ap())
            nc.gpsimd.dma_start(bqa_t[:], bqa_d.ap())
            nc.gpsimd.dma_start(bqb_t[:], bqb_d.ap())
            nc.gpsimd.dma_start(maska_t[:], maska_d.ap())
            nc.gpsimd.dma_start(maskb_t[:], maskb_d.ap())
            for j in range(2):
                nc.gpsimd.dma_start(bk_t[j][:], bk_d.ap()[j * 128 : (j + 1) * 128, :])
            nc.gpsimd.dma_start(bv_t[:], bv_d.ap())
            nc.gpsimd.dma_start(bp_t[:], bp_d.ap())

            # ---- PE warm-up + ACT exp-table preload during DMA ramp ----
            with tc.tile_pool(name="warm", bufs=1, space="PSUM") as warm_pool:
                wps = warm_pool.tile([128, 128], FP32, tag="warm")
                for w in range(32):
                    nc.tensor.matmul(
                        wps[:], ones_t[:], ones_t[:],
                        start=True, stop=True, skip_group_check=True,
                    )
            nc.scalar.activation(
                actw_t[:], maska_t[:], mybir.ActivationFunctionType.Exp, scale=0.125
            )

            with (
                tc.tile_pool(name="xp", bufs=1) as x_pool,
                tc.tile_pool(name="ktown", bufs=1) as ktown_pool,
                tc.tile_pool(name="vsb", bufs=1) as vsb_pool,
                tc.tile_pool(name="psA", bufs=1, space="PSUM") as psA,
            ):
                x_t = [x_pool.tile([128, S], BF16, name=f"x{k}", tag=f"x{k}")
                       for k in range(N_KT)]
                xq_t = [x_pool.tile([128, S_OWN], BF16, name=f"xq{k}", tag=f"xq{k}")
                        for k in range(N_KT)]
                kT_own = [ktown_pool.tile([128, S], BF16, name=f"ko{j}", tag=f"ko{j}")
                          for j in range(2)]
                vsb = [vsb_pool.tile([128, D], BF16, name=f"vsb{m}", tag=f"vsb{m}")
                       for m in range(4)]

                # input streams: x on sync, weights on scalar
                for k in range(N_KT):
                    sl = slice(k * 128, (k + 1) * 128)
                    nc.sync.dma_start(x_t[k][:], xT_d.ap()[sl, :])
                    nc.sync.dma_start(xq_t[k][:], xq_d.ap()[sl, :])
                    nc.scalar.dma_start(wk_t[k][:], wk_d.ap()[sl, :])
                for k in range(N_KT):
                    sl = slice(k * 128, (k + 1) * 128)
                    nc.scalar.dma_start(wv_t[k][:], wv_d.ap()[sl, :])
                for k in range(N_KT):
                    sl = slice(k * 128, (k + 1) * 128)
                    nc.scalar.dma_start(wq_t[k][:], wq_d.ap()[sl, :])

                ps = [psA.tile([128, 512], FP32, name=f"psA{g}", tag=f"psA{g}")
                      for g in range(8)]

                # -- K wave: kT_own[j2][:, sc] = wk.T @ xT ---------------
                for k in range(N_KT):
                    for j2 in range(2):
                        for sc in range(4):
                            nc.tensor.matmul(
                                ps[j2 * 4 + sc][:],
                                wk_t[k][:, j2 * 128 : (j2 + 1) * 128],
                                x_t[k][:, sc * 512 : (sc + 1) * 512],
                                start=(k == 0), stop=(k == N_KT - 1),
                            )
                for j2 in range(2):
                    for sc in range(4):
                        nc.vector.tensor_scalar(
                            kT_own[j2][:, sc * 512 : (sc + 1) * 512],
                            ps[j2 * 4 + sc][:], bk_t[j2][:], None,
                            mybir.AluOpType.add,
                        )
                for j2 in range(2):
                    nc.sync.dma_start(
                        agk_in.ap()[j2 * 128 : (j2 + 1) * 128, :], kT_own[j2][:]
                    )
                nc.gpsimd.collective_compute(
                    "AllGather", mybir.AluOpType.bypass, replica_groups=groups,
                    ins=[agk_in.ap()], outs=[agk_out.ap()],
                )

                # -- V wave: V_own[m][:, nb] = x_own @ wv ----------------
                ps_v = [psA.tile([128, 512], FP32, name=f"psV{g}", tag=f"psA{g}")
                        for g in range(8)]
                for k in range(N_KT):
                    for m in range(4):
                        for nb in range(2):
                            nc.tensor.matmul(
                                ps_v[m * 2 + nb][:],
                                xq_t[k][:, m * 128 : (m + 1) * 128],
                                wv_t[k][:, nb * 512 : (nb + 1) * 512],
                                start=(k == 0), stop=(k == N_KT - 1),
                            )
                for m in range(4):
                    for nb in range(2):
                        nc.vector.tensor_tensor(
                            vsb[m][:, nb * 512 : (nb + 1) * 512],
                            ps_v[m * 2 + nb][:],
                            bv_t[:, nb * 512 : (nb + 1) * 512],
                            mybir.AluOpType.add,
                        )
                for m in range(4):
                    nc.sync.dma_start(
                        agv_in.ap()[m * 128 : (m + 1) * 128, :], vsb[m][:]
                    )
                nc.gpsimd.collective_compute(
                    "AllGather", mybir.AluOpType.bypass, replica_groups=groups,
                    ins=[agv_in.ap()], outs=[agv_out.ap()],
                )

                # -- Q wave: qT_own[j] = wq.T @ x_own.T, masked halves ---
                ps_q = [psA.tile([128, 512], FP32, name=f"psQ{g}", tag=f"psA{g}")
                        for g in range(8)]
                for k in range(N_KT):
                    for j in range(8):
                        nc.tensor.matmul(
                            ps_q[j][:],
                            wq_t[k][:, j * 128 : (j + 1) * 128],
                            xq_t[k][:],
                            start=(k == 0), stop=(k == N_KT - 1),
                        )
                for j in range(8):
                    nc.vector.tensor_scalar(
                        qA[j][:], ps_q[j][:], maska_t[:], bqa_t[:, j : j + 1],
                        mybir.AluOpType.mult, mybir.AluOpType.add,
                    )
                    nc.vector.tensor_scalar(
                        qB[j][:], ps_q[j][:], maskb_t[:], bqb_t[:, j : j + 1],
                        mybir.AluOpType.mult, mybir.AluOpType.add,
                    )

                # gathered-data loads (wait on the collectives; sync +
                # vector queues are otherwise idle by now)
                for p in range(PAIRS):
                    nc.sync.dma_start(
                        kT[p][:], agk_out.ap()[p * 128 : (p + 1) * 128, :]
                    )
                for m in range(N_SK):
                    nc.gpsimd.dma_start(vp[m][:, :, HD : HD + 1], ones16_d.ap())
                for m in range(N_SK):
                    eng = nc.sync if m % 2 == 0 else nc.gpsimd
                    msl = slice(m * 128, (m + 1) * 128)
                    for j in range(GROUP):
                        eng.dma_start(
                            vp[m][:, j * LH : (j + 1) * LH, 0:HD],
                            agv_out.ap()[msl, j * LHD : (j + 1) * LHD],
                        )

            # wp prefetch (rotates into the w1024 slots wq used)
            wp_t = [w_pool.tile([128, D], BF16, name=f"wp{k}", tag="w1024", bufs=16)
                    for k in range(N_KT)]
            for k in range(N_KT):
                nc.scalar.dma_start(wp_t[k][:], wp_d.ap()[k * 128 : (k + 1) * 128, :])

            # ---- phase B: attention ------------------------------------
            with (
                tc.tile_pool(name="p", bufs=8) as p_pool,
                tc.tile_pool(name="rr", bufs=4) as rr_pool,
                tc.tile_pool(name="rcp", bufs=4) as rcp_pool,
                tc.tile_pool(name="psc", bufs=3, space="PSUM") as ps_sc,
                tc.tile_pool(name="pacc", bufs=1, space="PSUM") as ps_acc,
            ):
                for p in range(PAIRS):
                    acc_a = ps_acc.tile([128, S_OWN], FP32, tag="acca")
                    acc_b = ps_acc.tile([128, S_OWN], FP32, tag="accb")
                    for tp in range(N_SK // 2):
                        t0, t1 = 2 * tp, 2 * tp + 1
                        sca = ps_sc.tile([128, 1024], FP32, tag="sc")
                        scb = ps_sc.tile([128, 1024], FP32, tag="sc")
                        for ti, t in enumerate((t0, t1)):
                            tsl = slice(t * 128, (t + 1) * 128)
                            usl = slice(ti * 512, (ti + 1) * 512)
                            nc.tensor.matmul(
                                sca[:, usl], kT[p][:, tsl], qA[p][:],
                                start=True, stop=True,
                            )
                            nc.tensor.matmul(
                                scb[:, usl], kT[p][:, tsl], qB[p][:],
                                start=True, stop=True,
                            )
                        pa = p_pool.tile([128, 1024], BF16, tag="pt")
                        pb = p_pool.tile([128, 1024], BF16, tag="pt")
                        nc.scalar.activation(
                            pa[:], sca[:],
                            mybir.ActivationFunctionType.Exp, scale=0.125,
                        )
                        nc.scalar.activation(
                            pb[:], scb[:],
                            mybir.ActivationFunctionType.Exp, scale=0.125,
                        )
                        for ti, t in enumerate((t0, t1)):
                            usl = slice(ti * 512, (ti + 1) * 512)
                            first = tp == 0 and ti == 0
                            last = tp == N_SK // 2 - 1 and ti == 1
                            nc.tensor.matmul(
                                acc_a[0:65, :],
                                vp[t][:, 2 * p : 2 * p + 1, :],
                                pa[:, usl],
                                start=first, stop=last,
                            )
                            nc.tensor.matmul(
                                acc_b[0:65, :],
                                vp[t][:, 2 * p + 1 : 2 * p + 2, :],
                                pb[:, usl],
                                start=first, stop=last,
                            )
                    # normalize: 1/rowsum broadcast over the 64 head dims
                    for acc, half in ((acc_a, 0), (acc_b, 1)):
                        rrow = rr_pool.tile([1, S_OWN], FP32R, tag="rrow")
                        nc.vector.tensor_copy(rrow[:], acc[64:65, :])
                        rb = ps_sc.tile([64, S_OWN], FP32, tag="sc")
                        nc.tensor.matmul(
                            rb[:], ones_t[0:1, 0:64], rrow[:],
                            start=True, stop=True,
                        )
                        rc = rcp_pool.tile([64, S_OWN], FP32, tag="rc")
                        nc.vector.reciprocal_approx_fast(rc[:], rb[:])
                        nc.vector.tensor_tensor(
                            aoT[p][64 * half : 64 * half + 64, :],
                            acc[0:64, :], rc[:],
                            mybir.AluOpType.mult,
                        )

            # ---- phase C: local output projection ----------------------
            with (
                tc.tile_pool(name="outp", bufs=4) as out_pool,
                tc.tile_pool(name="psD", bufs=1, space="PSUM") as psD,
            ):
                for m in range(4):
                    msl = slice(m * 128, (m + 1) * 128)
                    for nb in range(2):
                        nsl = slice(nb * 512, (nb + 1) * 512)
                        pd = psD.tile([128, 512], FP32, tag=f"psD{m * 2 + nb}")
                        for kd in range(N_KT):
                            nc.tensor.matmul(
                                pd[:],
                                aoT[kd][:, msl],
                                wp_t[kd][:, nsl],
                                start=(kd == 0), stop=(kd == N_KT - 1),
                            )
                        ot = out_pool.tile([128, 512], FP32, tag="ot")
                        nc.vector.tensor_tensor(
                            ot[:], pd[:], bp_t[:, nsl], mybir.AluOpType.add
                        )
                        nc.sync.dma_start(out_d.ap()[msl, nsl], ot[:])

    nc.compile()
    return nc


def _get_program():
    global _compiled
    if _compiled is None:
        _compiled = _build()
    return _compiled


def _make_in_maps(x, w_qkv, b_qkv, w_proj, b_proj):
    x = np.asarray(x, dtype=np.float32)
    w_qkv = np.asarray(w_qkv, dtype=np.float32)
    b_qkv = np.asarray(b_qkv, dtype=np.float32)
    w_proj = np.asarray(w_proj, dtype=np.float32)
    b_proj = np.asarray(b_proj, dtype=np.float32)

    global _ONES16
    if _ONES16 is None:
        _ONES16 = np.ones((128, H), dtype=BF16_NP)

    wq16 = np.ascontiguousarray(w_qkv[:, 0:D]).astype(BF16_NP)
    wv_f = w_qkv[:, 2 * D : 3 * D]
    wv16 = np.ascontiguousarray(wv_f).astype(BF16_NP)
    wp16 = w_proj.astype(BF16_NP)
    bq = b_qkv[0:D]
    bqa = np.zeros((128, PAIRS), dtype=np.float32)
    bqb = np.zeros((128, PAIRS), dtype=np.float32)
    for j in range(PAIRS):
        bqa[0:64, j] = bq[j * 128 : j * 128 + 64]
        bqb[64:128, j] = bq[j * 128 + 64 : (j + 1) * 128]
    maska = np.zeros((128, 1), dtype=np.float32)
    maska[0:64] = 1.0
    maskb = 1.0 - maska
    bv_b = np.ascontiguousarray(
        np.broadcast_to(b_qkv[2 * D : 3 * D].reshape(1, D), (128, D))
    )
    bp_b = np.ascontiguousarray(np.broadcast_to(b_proj.reshape(1, D), (128, D)))

    xT = [np.ascontiguousarray(x[g].T).astype(BF16_NP) for g in range(B)]
    in_maps = []
    for c in range(N_CORES):
        g, r = c // GROUP, c % GROUP
        in_maps.append(
            {
                "xT": xT[g],
                "xq": np.ascontiguousarray(
                    xT[g][:, r * S_OWN : (r + 1) * S_OWN]
                ),
                "wq": wq16,
                "wk": np.ascontiguousarray(
                    w_qkv[:, D + r * LHD : D + (r + 1) * LHD]
                ).astype(BF16_NP),
                "wv": wv16,
                "wp": wp16,
                "bqa": bqa,
                "bqb": bqb,
                "maska": maska,
                "maskb": maskb,
                "bk": np.ascontiguousarray(
                    b_qkv[D + r * LHD : D + (r + 1) * LHD].reshape(LHD, 1)
                ),
                "bv": bv_b,
                "bp": bp_b,
                "ones": _ONES,
                "ones16": _ONES16,
            }
        )
    return in_maps


def _assemble(results):
    out = np.empty((B, S, D), dtype=np.float32)
    for c in range(N_CORES):
        g, r = c // GROUP, c % GROUP
        out[g, r * S_OWN : (r + 1) * S_OWN, :] = results[c]["out"]
    return out


def kernel(x, w_qkv, b_qkv, w_proj, b_proj):
    nc = _get_program()
    in_maps = _make_in_maps(x, w_qkv, b_qkv, w_proj, b_proj)
    res = run_bass_kernel_spmd(nc, in_maps, list(range(N_CORES)))
    return _assemble(res.results)


# revision 5
# speedup vs baseline: 1.1602x; 1.1152x over previous
"""Multi-head attention (B=2, S=2048, D=1024, H=16) on 8 Trainium2 NeuronCores.

Sharding: data-parallel over batch (groups of 4 cores) x sequence-parallel
attention inside each group.
  core c -> batch g = c // 4, sequence block r = c % 4 (rows r*512..r*512+512).

Per core (own rows = own 512 queries = own 512 keys):
  phase A (projections; every matmul is K=128 N=512 off the same xq tiles):
    kT_part [1024, 512] = wq-style wk_full.T @ xq   (K, all heads, own keys)
    V_own   [512, 1024] = xq.T @ wv_full            (V, all heads, own rows)
    qT_own  [1024, 512] = wq_full.T @ xq            (Q, all heads, own queries)
  Two early AllGathers per 4-core group, triggered as soon as their producer
  wave finishes: K first (it feeds the long exp chain), then V.
  Key tiles are RELABELED per core (block b = group-rank (r+b)%4) so tiles
  0..3 are always the core's own keys: attention on them needs no gather and
  the SPMD program stays compile-time static; softmax is order-invariant.
  phase B (attention, all 16 heads x own 512 queries x 2048 keys):
    per head pair, per local key-tile pair: scoresT = kT-tile.T @ q with the
    q halves zero-padded per head (K=128, shared stationary operand), one
    N=1024 exp per head on ScalarE, PV accumulate with an appended ones
    column collecting the softmax denominator (row 64 of the PSUM acc).
  phase C: local output projection out = aoT.T @ w_proj + b_proj (no
  collective on the tail).
Host-side work is only slicing/transposition of inputs and concatenation of
outputs.
"""

import os
import sys

import numpy as np

try:
    import ml_dtypes

    BF16_NP = ml_dtypes.bfloat16
except ImportError:  # pragma: no cover
    BF16_NP = None

for _p in ("/opt/trn_rl_repo",):
    if os.path.isdir(_p) and _p not in sys.path:
        sys.path.append(_p)

import concourse.bass as bass  # noqa: E402
import concourse.mybir as mybir  # noqa: E402
import concourse.tile as tile  # noqa: E402
from concourse import bacc  # noqa: E402
from concourse.bass_utils import run_bass_kernel_spmd  # noqa: E402

B, S, D = 2, 2048, 1024
H, HD = 16, 64
N_CORES = 8
GROUP = 4
S_OWN = S // GROUP  # 512 own rows (queries and keys)
N_KT = D // 128  # 8 contraction tiles
N_SK = S // 128  # 16 key tiles
PAIRS = H // 2  # 8 head pairs

FP32 = mybir.dt.float32
FP32R = mybir.dt.float32r
BF16 = mybir.dt.bfloat16

_compiled = None
_ONES = np.ones((128, 128), dtype=np.float32)
_ONES16 = None

# packed f32 per-partition constants: [bqa 0:8 | bqb 8:16 | bk 16:24 |
#  maska 24 | maskb 25]
FC_BQA, FC_BQB, FC_BK, FC_MA, FC_MB, FC_W = 0, 8, 16, 24, 25, 26


def _build():
    nc = bacc.Bacc(
        "TRN2", target_bir_lowering=False, debug=False, num_devices=N_CORES
    )

    xq_d = nc.dram_tensor("xq", [D, S_OWN], BF16, kind="ExternalInput")
    wq_d = nc.dram_tensor("wq", [D, D], BF16, kind="ExternalInput")
    wk_d = nc.dram_tensor("wk", [D, D], BF16, kind="ExternalInput")
    wv_d = nc.dram_tensor("wv", [D, D], BF16, kind="ExternalInput")
    wp_d = nc.dram_tensor("wp", [D, D], BF16, kind="ExternalInput")
    fc_d = nc.dram_tensor("fc", [128, FC_W], FP32, kind="ExternalInput")
    bv_d = nc.dram_tensor("bv", [128, D], FP32, kind="ExternalInput")
    bp_d = nc.dram_tensor("bp", [128, D], FP32, kind="ExternalInput")
    ones_d = nc.dram_tensor("ones", [128, 128], FP32R, kind="ExternalInput")
    ones16_d = nc.dram_tensor("ones16", [128, H], BF16, kind="ExternalInput")
    out_d = nc.dram_tensor("out", [S_OWN, D], FP32, kind="ExternalOutput")

    agk_in = nc.dram_tensor("agk_in", [D, S_OWN], BF16)
    agk_out = nc.dram_tensor("agk_out", [GROUP * D, S_OWN], BF16)
    agv_in = nc.dram_tensor("agv_in", [S_OWN, D], BF16)
    agv_out = nc.dram_tensor("agv_out", [S, D], BF16)
    groups = [[0, 1, 2, 3], [4, 5, 6, 7]]

    with tile.TileContext(nc) as tc:
        import contextlib

        with contextlib.ExitStack() as stk:
            # ---- persistent pools --------------------------------------
            w_pool = stk.enter_context(tc.tile_pool(name="w", bufs=1))
            kt_pool = stk.enter_context(tc.tile_pool(name="kt", bufs=1))
            vp_pool = stk.enter_context(tc.tile_pool(name="vp", bufs=1))
            q_pool = stk.enter_context(tc.tile_pool(name="q", bufs=1))
            ao_pool = stk.enter_context(tc.tile_pool(name="ao", bufs=1))
            const_pool = stk.enter_context(tc.tile_pool(name="const", bufs=1))

            # 1024-wide weight tiles: wk(8)+wv(8)+wq(8) live in phase A;
            # wp(8) rotates into wk's slots afterwards
            wk_t = [w_pool.tile([128, D], BF16, name=f"wk{k}", tag="w1024", bufs=24)
                    for k in range(N_KT)]
            wv_t = [w_pool.tile([128, D], BF16, name=f"wv{k}", tag="w1024", bufs=24)
                    for k in range(N_KT)]
            wq_t = [w_pool.tile([128, D], BF16, name=f"wq{k}", tag="w1024", bufs=24)
                    for k in range(N_KT)]

            kT = [kt_pool.tile([128, S], BF16, name=f"kT{p}", tag=f"kT{p}")
                  for p in range(PAIRS)]
            vp = [vp_pool.tile([128, H, HD + 1], BF16, name=f"vp{m}", tag=f"vp{m}")
                  for m in range(N_SK)]
            qA = [q_pool.tile([128, S_OWN], BF16, name=f"qA{p}", tag=f"qA{p}")
                  for p in range(PAIRS)]
            qB = [q_pool.tile([128, S_OWN], BF16, name=f"qB{p}", tag=f"qB{p}")
                  for p in range(PAIRS)]
            aoT = [ao_pool.tile([128, S_OWN], BF16, name=f"ao{p}", tag=f"ao{p}")
                   for p in range(PAIRS)]

            ones_t = const_pool.tile([128, 128], FP32R, tag="ones")
            ones16_t = const_pool.tile([128, H], BF16, tag="ones16")
            fc_t = const_pool.tile([128, FC_W], FP32, tag="fc")
            bv_t = const_pool.tile([128, D], FP32, tag="bv")
            bp_t = const_pool.tile([128, D], FP32, tag="bp")
            actw_t = const_pool.tile([128, 1], FP32, tag="actw")

            nc.gpsimd.dma_start(ones_t[:], ones_d.ap())
            nc.gpsimd.dma_start(fc_t[:], fc_d.ap())
            nc.gpsimd.dma_start(ones16_t[:], ones16_d.ap())

            # ---- PE warm-up + ACT exp-table preload during DMA ramp ----
            with tc.tile_pool(name="warm", bufs=1, space="PSUM") as warm_pool:
                wps = warm_pool.tile([128, 128], FP32, tag="warm")
                for w in range(32):
                    nc.tensor.matmul(
                        wps[:], ones_t[:], ones_t[:],
                        start=True, stop=True, skip_group_check=True,
                    )
            with (
                tc.tile_pool(name="xp", bufs=1) as x_pool,
                tc.tile_pool(name="vsb", bufs=1) as vsb_pool,
                tc.tile_pool(name="vtmp", bufs=4) as vtmp_pool,
                tc.tile_pool(name="psA", bufs=1, space="PSUM") as psA,
            ):
                xq_t = [x_pool.tile([128, S_OWN], BF16, name=f"xq{k}", tag=f"xq{k}")
                        for k in range(N_KT)]
                vsb = [vsb_pool.tile([128, D], BF16, name=f"vsb{m}", tag=f"vsb{m}")
                       for m in range(4)]

                # input streams: xq + wv on sync, wk + wq (+bv/bp, wp) on scalar
                for k in range(N_KT):
                    sl = slice(k * 128, (k + 1) * 128)
                    nc.sync.dma_start(xq_t[k][:], xq_d.ap()[sl, :])
                    nc.scalar.dma_start(wk_t[k][:], wk_d.ap()[sl, :])
                for k in range(N_KT):
                    sl = slice(k * 128, (k + 1) * 128)
                    nc.sync.dma_start(wv_t[k][:], wv_d.ap()[sl, :])
                    nc.scalar.dma_start(wq_t[k][:], wq_d.ap()[sl, :])
                nc.scalar.dma_start(bv_t[:], bv_d.ap())
                nc.scalar.dma_start(bp_t[:], bp_d.ap())
                nc.scalar.activation(
                    actw_t[:], fc_t[:, FC_MA : FC_MA + 1],
                    mybir.ActivationFunctionType.Exp, scale=0.125,
                )

                ps = [psA.tile([128, 512], FP32, name=f"psA{g}", tag=f"psA{g}")
                      for g in range(8)]

                # -- K wave: kT_part[j] = wk.T @ xq; evac into kT[j][:, 0:512]
                for k in range(N_KT):
                    for j in range(8):
                        nc.tensor.matmul(
                            ps[j][:],
                            wk_t[k][:, j * 128 : (j + 1) * 128],
                            xq_t[k][:],
                            start=(k == 0), stop=(k == N_KT - 1),
                        )
                for j in range(8):
                    nc.vector.tensor_scalar(
                        kT[j][:, 0:S_OWN], ps[j][:],
                        fc_t[:, FC_BK + j : FC_BK + j + 1], None,
                        mybir.AluOpType.add,
                    )
                for j in range(8):
                    nc.sync.dma_start(
                        agk_in.ap()[j * 128 : (j + 1) * 128, :], kT[j][:, 0:S_OWN]
                    )
                nc.gpsimd.collective_compute(
                    "AllGather", mybir.AluOpType.bypass, replica_groups=groups,
                    ins=[agk_in.ap()], outs=[agk_out.ap()],
                )

                # -- V wave: V_own[m] = xq.T @ wv -------------------------
                ps_v = [psA.tile([128, 512], FP32, name=f"psV{g}", tag=f"psA{g}")
                        for g in range(8)]
                for k in range(N_KT):
                    for m in range(4):
                        for nb in range(2):
                            nc.tensor.matmul(
                                ps_v[m * 2 + nb][:],
                                xq_t[k][:, m * 128 : (m + 1) * 128],
                                wv_t[k][:, nb * 512 : (nb + 1) * 512],
                                start=(k == 0), stop=(k == N_KT - 1),
                            )
                for m in range(4):
                    for nb in range(2):
                        nc.vector.tensor_tensor(
                            vsb[m][:, nb * 512 : (nb + 1) * 512],
                            ps_v[m * 2 + nb][:],
                            bv_t[:, nb * 512 : (nb + 1) * 512],
                            mybir.AluOpType.add,
                        )
                for m in range(4):
                    nc.sync.dma_start(
                        agv_in.ap()[m * 128 : (m + 1) * 128, :], vsb[m][:]
                    )
                nc.gpsimd.collective_compute(
                    "AllGather", mybir.AluOpType.bypass, replica_groups=groups,
                    ins=[agv_in.ap()], outs=[agv_out.ap()],
                )

                # own V (local key tiles 0..3): interleave via DVE, no DMA
                for m in range(4):
                    nc.vector.tensor_copy(vp[m][:, :, HD : HD + 1], ones16_t[:])
                    nc.vector.tensor_copy(vp[m][:, :, 0:HD], vsb[m][:])

                # -- Q wave: qT_own[j] = wq.T @ xq, masked/padded halves --
                ps_q = [psA.tile([128, 512], FP32, name=f"psQ{g}", tag=f"psA{g}")
                        for g in range(8)]
                for k in range(N_KT):
                    for j in range(8):
                        nc.tensor.matmul(
                            ps_q[j][:],
                            wq_t[k][:, j * 128 : (j + 1) * 128],
                            xq_t[k][:],
                            start=(k == 0), stop=(k == N_KT - 1),
                        )
                for j in range(8):
                    nc.vector.tensor_scalar(
                        qA[j][:], ps_q[j][:],
                        fc_t[:, FC_MA : FC_MA + 1],
                        fc_t[:, FC_BQA + j : FC_BQA + j + 1],
                        mybir.AluOpType.mult, mybir.AluOpType.add,
                    )
                    nc.vector.tensor_scalar(
                        qB[j][:], ps_q[j][:],
                        fc_t[:, FC_MB : FC_MB + 1],
                        fc_t[:, FC_BQB + j : FC_BQB + j + 1],
                        mybir.AluOpType.mult, mybir.AluOpType.add,
                    )

                # -- gathered loads (runtime-rotated so tiles 0..3 = own) --
                pid = nc.gpsimd.partition_id()
                rank = pid % GROUP
                # kT[p] key block b (b>=1) from group-rank (r+b)%4
                for p in range(PAIRS):
                    for b in range(1, GROUP):
                        row0 = ((rank + b) % GROUP) * D + p * 128
                        nc.gpsimd.dma_start(
                            kT[p][:, b * S_OWN : (b + 1) * S_OWN],
                            agk_out.ap()[bass.ds(row0, 128), :],
                        )
                # V blocks b>=1: contiguous DMA then DVE interleave
                for b in range(1, GROUP):
                    for i in range(4):
                        m = b * 4 + i
                        vt = vtmp_pool.tile([128, D], BF16, name=f"vt{m}", tag="vt")
                        row0 = ((rank + b) % GROUP) * S_OWN + i * 128
                        nc.gpsimd.dma_start(
                            vt[:], agv_out.ap()[bass.ds(row0, 128), :]
                        )
                        nc.vector.tensor_copy(vp[m][:, :, HD : HD + 1], ones16_t[:])
                        nc.vector.tensor_copy(vp[m][:, :, 0:HD], vt[:])

            # wp prefetch (rotates into w1024 slots)
            wp_t = [w_pool.tile([128, D], BF16, name=f"wp{k}", tag="w1024", bufs=24)
                    for k in range(N_KT)]
            for k in range(N_KT):
                nc.scalar.dma_start(wp_t[k][:], wp_d.ap()[k * 128 : (k + 1) * 128, :])

            # ---- phase B: attention ------------------------------------
            with (
                tc.tile_pool(name="p", bufs=8) as p_pool,
                tc.tile_pool(name="rr", bufs=4) as rr_pool,
                tc.tile_pool(name="rcp", bufs=4) as rcp_pool,
                tc.tile_pool(name="psc", bufs=2, space="PSUM") as ps_sc,
                tc.tile_pool(name="pacc", bufs=2, space="PSUM") as ps_acc,
            ):
                for p in range(PAIRS):
                    acc_a = ps_acc.tile([128, S_OWN], FP32, tag="acca")
                    acc_b = ps_acc.tile([128, S_OWN], FP32, tag="accb")
                    for tp in range(N_SK // 2):
                        t0, t1 = 2 * tp, 2 * tp + 1
                        sca = ps_sc.tile([128, 1024], FP32, tag="sc")
                        scb = ps_sc.tile([128, 1024], FP32, tag="sc")
                        for ti, t in enumerate((t0, t1)):
                            tsl = slice(t * 128, (t + 1) * 128)
                            usl = slice(ti * 512, (ti + 1) * 512)
                            nc.tensor.matmul(
                                sca[:, usl], kT[p][:, tsl], qA[p][:],
                                start=True, stop=True,
                            )
                            nc.tensor.matmul(
                                scb[:, usl], kT[p][:, tsl], qB[p][:],
                                start=True, stop=True,
                            )
                        pa = p_pool.tile([128, 1024], BF16, tag="pt")
                        pb = p_pool.tile([128, 1024], BF16, tag="pt")
                        nc.scalar.activation(
                            pa[:], sca[:],
                            mybir.ActivationFunctionType.Exp, scale=0.125,
                        )
                        nc.scalar.activation(
                            pb[:], scb[:],
                            mybir.ActivationFunctionType.Exp, scale=0.125,
                        )
                        for ti, t in enumerate((t0, t1)):
                            usl = slice(ti * 512, (ti + 1) * 512)
                            first = tp == 0 and ti == 0
                            last = tp == N_SK // 2 - 1 and ti == 1
                            nc.tensor.matmul(
                                acc_a[0:65, :],
                                vp[t][:, 2 * p : 2 * p + 1, :],
                                pa[:, usl],
                                start=first, stop=last,
                            )
                            nc.tensor.matmul(
                                acc_b[0:65, :],
                                vp[t][:, 2 * p + 1 : 2 * p + 2, :],
                                pb[:, usl],
                                start=first, stop=last,
                            )
                    # normalize: 1/rowsum broadcast over the 64 head dims
                    for acc, half in ((acc_a, 0), (acc_b, 1)):
                        rrow = rr_pool.tile([1, S_OWN], FP32R, tag="rrow")
                        nc.vector.tensor_copy(rrow[:], acc[64:65, :])
                        rb = ps_sc.tile([64, S_OWN], FP32, tag="sc")
                        nc.tensor.matmul(
                            rb[:], ones_t[0:1, 0:64], rrow[:],
                            start=True, stop=True,
                        )
                        rc = rcp_pool.tile([64, S_OWN], FP32, tag="rc")
                        nc.vector.reciprocal_approx_fast(rc[:], rb[:])
                        nc.vector.tensor_tensor(
                            aoT[p][64 * half : 64 * half + 64, :],
                            acc[0:64, :], rc[:],
                            mybir.AluOpType.mult,
                        )

            # ---- phase C: local output projection ----------------------
            with (
                tc.tile_pool(name="outp", bufs=4) as out_pool,
                tc.tile_pool(name="psD", bufs=1, space="PSUM") as psD,
            ):
                for m in range(4):
                    msl = slice(m * 128, (m + 1) * 128)
                    for nb in range(2):
                        nsl = slice(nb * 512, (nb + 1) * 512)
                        pd = psD.tile([128, 512], FP32, tag=f"psD{m * 2 + nb}")
                        for kd in range(N_KT):
                            nc.tensor.matmul(
                                pd[:],
                                aoT[kd][:, msl],
                                wp_t[kd][:, nsl],
                                start=(kd == 0), stop=(kd == N_KT - 1),
                            )
                        ot = out_pool.tile([128, 512], FP32, tag="ot")
                        nc.vector.tensor_tensor(
                            ot[:], pd[:], bp_t[:, nsl], mybir.AluOpType.add
                        )
                        nc.sync.dma_start(out_d.ap()[msl, nsl], ot[:])

    nc.compile()
    return nc


def _get_program():
    global _compiled
    if _compiled is None:
        _compiled = _build()
    return _compiled


def _make_in_maps(x, w_qkv, b_qkv, w_proj, b_proj):
    x = np.asarray(x, dtype=np.float32)
    w_qkv = np.asarray(w_qkv, dtype=np.float32)
    b_qkv = np.asarray(b_qkv, dtype=np.float32)
    w_proj = np.asarray(w_proj, dtype=np.float32)
    b_proj = np.asarray(b_proj, dtype=np.float32)

    global _ONES16
    if _ONES16 is None:
        _ONES16 = np.ones((128, H), dtype=BF16_NP)

    wq16 = np.ascontiguousarray(w_qkv[:, 0:D]).astype(BF16_NP)
    wk16 = np.ascontiguousarray(w_qkv[:, D : 2 * D]).astype(BF16_NP)
    wv16 = np.ascontiguousarray(w_qkv[:, 2 * D : 3 * D]).astype(BF16_NP)
    wp16 = w_proj.astype(BF16_NP)

    fc = np.zeros((128, FC_W), dtype=np.float32)
    bq = b_qkv[0:D]
    bk = b_qkv[D : 2 * D]
    for j in range(PAIRS):
        fc[0:64, FC_BQA + j] = bq[j * 128 : j * 128 + 64]
        fc[64:128, FC_BQB + j] = bq[j * 128 + 64 : (j + 1) * 128]
        fc[:, FC_BK + j] = bk[j * 128 : (j + 1) * 128]
    fc[0:64, FC_MA] = 1.0
    fc[64:128, FC_MB] = 1.0
    bv_b = np.ascontiguousarray(
        np.broadcast_to(b_qkv[2 * D : 3 * D].reshape(1, D), (128, D))
    )
    bp_b = np.ascontiguousarray(np.broadcast_to(b_proj.reshape(1, D), (128, D)))

    xT = [np.ascontiguousarray(x[g].T).astype(BF16_NP) for g in range(B)]
    in_maps = []
    for c in range(N_CORES):
        g, r = c // GROUP, c % GROUP
        in_maps.append(
            {
                "xq": np.ascontiguousarray(
                    xT[g][:, r * S_OWN : (r + 1) * S_OWN]
                ),
                "wq": wq16,
                "wk": wk16,
                "wv": wv16,
                "wp": wp16,
                "fc": fc,
                "bv": bv_b,
                "bp": bp_b,
                "ones": _ONES,
                "ones16": _ONES16,
            }
        )
    return in_maps


def _assemble(results):
    out = np.empty((B, S, D), dtype=np.float32)
    for c in range(N_CORES):
        g, r = c // GROUP, c % GROUP
        out[g, r * S_OWN : (r + 1) * S_OWN, :] = results[c]["out"]
    return out


def kernel(x, w_qkv, b_qkv, w_proj, b_proj):
    nc = _get_program()
    in_maps = _make_in_maps(x, w_qkv, b_qkv, w_proj, b_proj)
    res = run_bass_kernel_spmd(nc, in_maps, list(range(N_CORES)))
    return _assemble(res.results)
